# revision 44
# baseline (speedup 1.0000x reference)
"""AR(24) extrapolation kernel for Trainium2 (8 NeuronCores, data parallel).

The reference runs a 168-step scalar-weight autoregressive recurrence over the
last 24 timesteps of x, independently per (batch, channel).  Because the
recurrence is linear, output step t is a fixed linear combination of the
initial 24-sample window plus a bias term:

    y[b, t, d] = sum_i C[i, t] * x[b, S-24+i, d] + beta[t]

C [24, 168] and beta [168] follow from W/b by unrolling the recurrence once on
the host (float64, ~4k flops).  The device work is then a memory-bound
broadcast matmul: per core, out[t, (b, d)] = CB^T @ xaug where CB stacks
[C; beta] and xaug stacks [x_window^T; ones].

Sharding: pure data parallelism over batch (32 batches per core).  Device
output layout is [T, B_loc*D] so DMA stores have 16KB-contiguous runs per
partition; the host transposes back when gathering.

Layout details per core:
- input xpack [100, 4096]: 32 moving blocks of [25, 512] (24 window rows + a
  ones row for the bias), block b at row-slot b%4, col-slot b//4.  100
  partitions -> 13 SDMA engines on loads instead of 4.
- matmul: stationary CB chunk [25, 84] (t split 84/84 so both output chunks
  cover partitions 0-83 evenly), moving [25, 512], PSUM [84, 512] fp32.
- float32r operands: fast fp32 path on the PE (full rate at N>=512).
- stores: one [84, 8*512] staging tile per (group, chunk); the two HWDGE
  rings (sync/scalar engines) alternate chunks per group.
"""

import numpy as np

import concourse.bass as bass
import concourse.bacc as bacc
import concourse.tile as tile
from concourse import mybir
from concourse.bass_utils import run_bass_kernel_spmd

ORDER = 24
K = ORDER + 1            # contraction: 24 window rows + ones row
T = 168
D = 512
B = 256
S = 336
N_CORES = 8
NB = B // N_CORES        # 32 local batches per core
COLS = NB * D            # 16384 columns per core
GROUPS = [4, 4, 8, 8, 8]  # batches per staged output group (small first so
                          # the store stream starts early, then steady-state)
assert sum(GROUPS) == NB
TC = T // 2              # 84: t-chunk size (two chunks cover partitions 0-83)
F32 = mybir.dt.float32
F32R = mybir.dt.float32r  # fast fp32 matmul path (full PE rate at N>=256)

_nc_cache = None


def _build_program():
    nc = bacc.Bacc()
    xp = nc.declare_dram_parameter("xpack", [128, (NB // 4) * D], F32R, isOutput=False)
    cb = nc.declare_dram_parameter("cb", [128, T], F32R, isOutput=False)
    # chunk0: t 0..127 as [t, (b, d)]; tail: t 128..167 stored transposed as
    # [d%128, (g, j, d//128, t-128)] so its stores cover all 128 partitions
    out = nc.declare_dram_parameter("out", [128, COLS], F32, isOutput=True)
    outt = nc.declare_dram_parameter(
        "outt", [128, NB * 4 * (T - 128)], F32, isOutput=True
    )

    with tile.TileContext(nc) as tc:
        with (
            tc.tile_pool(name="consts", bufs=1) as consts,
            tc.tile_pool(name="xin", bufs=1) as xin,
            tc.tile_pool(name="stage", bufs=4) as stage,
            tc.tile_pool(name="psum", bufs=4, space="PSUM") as psum,
        ):
            # input loads go first on the HWDGE rings (idle until stores
            # start); weights first since every matmul needs them
            cb_t = consts.tile([128, T], F32R)
            nc.sync.dma_start(out=cb_t, in_=cb[:, :])


            starts = [sum(GROUPS[:g]) for g in range(len(GROUPS))]
            xts = []
            for g, (b0, sz) in enumerate(zip(starts, GROUPS)):
                xt = xin.tile([128, sz * 128], F32R, tag=f"xt{g}")
                eng = nc.scalar if g % 2 == 0 else nc.sync
                eng.dma_start(out=xt, in_=xp[:, b0 * 128 : (b0 + sz) * 128])
                xts.append(xt)

            P0 = 128
            P1 = T - P0  # 40
            for g, (b0, sz) in enumerate(zip(starts, GROUPS)):
                c0 = b0 * D
                st0 = stage.tile([P0, sz * D], F32, tag="st0")
                st1 = stage.tile([P0, sz * 4 * P1], F32, tag="st1")
                for j in range(sz):
                    rs = 32 * (j % 4)
                    cs = (j // 4) * D
                    mv = xts[g][rs : rs + K, cs : cs + D]
                    wt0 = cb_t[rs : rs + K, 0:P0]
                    wtt = cb_t[rs : rs + K, P0:T]  # [K, 40] moving for the tail
                    ps0 = psum.tile([P0, D], F32, tag="ps0")
                    nc.tensor.matmul(
                        ps0, wt0, mv, start=True, stop=True, tile_position=(rs, 0)
                    )
                    nc.vector.tensor_copy(st0[:, j * D : (j + 1) * D], ps0)

                    # tail computed transposed: out[d_chunk, t'] = x_chunk^T @ C_tail
                    ps1 = psum.tile([P0, 4 * P1], F32, tag="ps1")
                    for q in range(4):
                        nc.tensor.matmul(
                            ps1[:, q * P1 : (q + 1) * P1],
                            xts[g][rs : rs + K, cs + 128 * q : cs + 128 * (q + 1)],
                            wtt,
                            start=True,
                            stop=True,
                            tile_position=(rs, 0),
                        )
                    nc.scalar.copy(
                        st1[:, j * 4 * P1 : (j + 1) * 4 * P1], ps1
                    )

                # alternate the two HWDGE rings (SP / Activation) per group so
                # store bandwidth and completion latency overlap
                eng0, eng1 = (nc.sync, nc.scalar) if g % 2 == 0 else (nc.scalar, nc.sync)
                eng0.dma_start(out=out[:, c0 : c0 + sz * D], in_=st0)
                eng1.dma_start(
                    out=outt[:, b0 * 4 * P1 : (b0 + sz) * 4 * P1], in_=st1
                )

    nc.finalize()
    return nc


def _unroll_coeffs(W: np.ndarray, b: np.ndarray) -> np.ndarray:
    """Unroll the linear AR recurrence: CB[k, t] with rows 0..23 = window
    coefficients, row 24 = additive bias per step."""
    w = W[:, 0].astype(np.float64)
    bb = float(np.asarray(b).reshape(-1)[0])
    M = np.eye(ORDER)
    m = np.zeros(ORDER)
    CB = np.zeros((K, T), np.float64)
    for t in range(T):
        c = M.T @ w
        yb = m @ w + bb
        CB[:ORDER, t] = c
        CB[ORDER, t] = yb
        M = np.vstack([M[1:], c[None, :]])
        m = np.concatenate([m[1:], [yb]])
    return CB.astype(np.float32)


def _pack_inputs(x: np.ndarray) -> np.ndarray:
    """Build per-core packed moving operands.

    Returns [N_CORES, 128, (NB//4)*D] where core c / block b (local batch)
    sits at row-slot b%4 (25 rows), col-slot b//4 (512 cols); block contents =
    [x[global_b, S-24+i, d] for i rows] plus a trailing ones row.
    """
    xw = x[:, -ORDER:, :]  # [B, 24, D]
    packed = np.zeros((N_CORES, 128, (NB // 4) * D), np.float32)
    for c in range(N_CORES):
        for b in range(NB):
            rs = 32 * (b % 4)
            cs = (b // 4) * D
            blk = xw[c * NB + b]  # [24, D]
            packed[c, rs : rs + ORDER, cs : cs + D] = blk
            packed[c, rs + ORDER, cs : cs + D] = 1.0
    return packed


def kernel(x, W, b, tar_seq_len):
    global _nc_cache
    x = np.asarray(x, dtype=np.float32)
    W = np.asarray(W, dtype=np.float32)
    b = np.asarray(b, dtype=np.float32)
    assert int(tar_seq_len) == T, f"compiled for tar_seq_len={T}"
    assert x.shape == (B, S, D)

    CB = _unroll_coeffs(W, b)
    packed = _pack_inputs(x)

    # replicate CB into each 32-row strip of the PE array (rows 25-31 zero)
    CBrep = np.zeros((128, T), np.float32)
    for s in range(4):
        CBrep[32 * s : 32 * s + K] = CB

    if _nc_cache is None:
        _nc_cache = _build_program()
    nc = _nc_cache

    in_maps = [{"xpack": packed[c], "cb": CBrep} for c in range(N_CORES)]
    res = run_bass_kernel_spmd(nc, in_maps, list(range(N_CORES)))

    # gather: chunk0 [128, NB*D] -> [NB, 128, D]; transposed tail
    # [128, (g, j, q, t')] -> [NB, 40, D] with d = 128*q + p
    P1 = T - 128
    parts = []
    for r in res.results:
        y = np.empty((NB, T, D), np.float32)
        y[:, 0:128, :] = r["out"].reshape(128, NB, D).transpose(1, 0, 2)
        tail = r["outt"].reshape(128, NB, 4, P1)
        y[:, 128:T, :] = tail.transpose(1, 3, 2, 0).reshape(NB, P1, D)
        parts.append(y)
    return np.ascontiguousarray(np.concatenate(parts, axis=0))
